# revision 10
# baseline (speedup 1.0000x reference)
"""CharRNN (highway-GRU, D=3, R=1024) Trainium2 Bass kernel.

Strategy: data-parallel over batch (B=128 -> 16 rows/core on 8 cores).
All recurrent weights stay SBUF-resident in fp16 (FWL-eligible), state
kept in f32, matmuls accumulate in f32 PSUM.  Everything in the
recurrence lives in [R-on-partitions x batch-on-free] layout so the
elementwise highway update runs across all 128 partitions.

Per core:
  head:  gather embedding rows (indirect DMA), PE-transpose to x^T,
         layer-0 input GEMM folded into the recurrence as two extra
         K-tiles of the gate matmuls.
  loop:  256 steps x 3 highway layers x 2 gates; W stationary
         [128,128] fp16 tiles, moving operand = s^T [128,16].
  tail:  batched output projection s_t @ Wp from the SBUF-resident
         state history, DMA out as out^T [256, 4096].
"""

import numpy as np

P = 128
B, T, R, NU, V, D = 128, 256, 1024, 256, 50000, 3
NCORES = 8
BC = B // NCORES          # 16 batch rows per core
KT = R // P               # 8 k-tiles over R
XK = NU // P              # 2 k-tiles over the embedding dim
NG = (BC * T) // P        # 32 gather tiles of 128 tokens
NTOK = BC * T             # 4096 tokens per core
DEBUG = False

_CACHED = {}


def _tile_layout(w, f16=True):
    """[K, M] -> [128, (K/128)*(M/128)*128] with column block (kt*MT+mt)*128
    holding w[kt*128:(kt+1)*128, mt*128:(mt+1)*128]."""
    K, M = w.shape
    kt, mt = K // P, M // P
    t = w.reshape(kt, P, mt, P).transpose(1, 0, 2, 3).reshape(P, kt * mt * P)
    return np.ascontiguousarray(t.astype(np.float16 if f16 else w.dtype))


def _uniform(b):
    b = np.asarray(b, np.float32)
    return float(b.flat[0]) if np.all(b == b.flat[0]) else None


def _build_program(bias_consts):
    """Build the SPMD Bass program. bias_consts: dict name->float|None.
    Non-uniform biases get a [128, ntiles] f32 input tensor and per-m-tile
    activation calls."""
    import concourse.bass as bass
    import concourse.tile as tile
    from concourse import bacc, mybir
    from concourse.masks import make_identity

    # stale ceiling: cayman has 208 KiB usable per partition
    try:
        import concourse.tile_utils as tile_utils
        tile_utils.max_sbuf_usage = 206 * 1024
    except Exception:
        pass

    f16, f32, i32 = mybir.dt.float16, mybir.dt.float32, mybir.dt.int32
    Act = mybir.ActivationFunctionType

    nc = bacc.Bacc("TRN2", target_bir_lowering=False, debug=False)

    emb = nc.dram_tensor("emb", [V, NU], f16, kind="ExternalInput").ap()
    idx = nc.dram_tensor("idx", [P, NG], i32, kind="ExternalInput").ap()
    wdr = {}
    for name, cols in [
        ("w0h", (XK + KT) * KT * P), ("w0t", (XK + KT) * KT * P),
        ("w1h", KT * KT * P), ("w1t", KT * KT * P),
        ("w2h", KT * KT * P), ("w2t", KT * KT * P),
        ("wp", KT * (NU // P) * P),
    ]:
        wdr[name] = nc.dram_tensor(name, [P, cols], f16, kind="ExternalInput").ap()
    bdr = {}
    for bn, nt in [("b0h", KT), ("b0t", KT), ("b1h", KT), ("b1t", KT),
                   ("b2h", KT), ("b2t", KT), ("bp", NU // P)]:
        if bias_consts[bn] is None:
            bdr[bn] = nc.dram_tensor(bn, [P, nt], f32, kind="ExternalInput").ap()
    out = nc.dram_tensor("out", [NU, NTOK], f32, kind="ExternalOutput").ap()
    dbg = {}
    if DEBUG:
        dbg["hist"] = nc.dram_tensor("dbg_hist", [P, T * P], f16,
                                     kind="ExternalOutput").ap()
        dbg["xT"] = nc.dram_tensor("dbg_xT", [P, XK * NTOK], f16,
                                   kind="ExternalOutput").ap()

    with tile.TileContext(nc) as tc:
        with tc.tile_pool(name="const", bufs=1) as const:
            idx_sb = const.tile([P, NG], i32)
            nc.sync.dma_start(idx_sb[:], idx[:])
            wsb = {}
            for name in wdr:
                wt = const.tile([P, wdr[name].shape[1]], f16, tag=name)
                wsb[name] = wt
                # split the load across queues
                cols = wdr[name].shape[1]
                nchunk = 4 if cols >= 8192 else 1
                cs = cols // nchunk
                for ci in range(nchunk):
                    nc.sync.dma_start(wt[:, ci * cs:(ci + 1) * cs],
                                      wdr[name][:, ci * cs:(ci + 1) * cs])
            bsb = {}
            for bn in bdr:
                bt = const.tile([P, bdr[bn].shape[1]], f32, tag="bias_" + bn)
                bsb[bn] = bt
                nc.sync.dma_start(bt[:], bdr[bn][:])
            ident = const.tile([P, P], f16)
            make_identity(nc, ident[:])
            # uniform nonzero bias constants as [P,1] tiles
            bconst = {}
            for bn, c in bias_consts.items():
                if c is not None and c != 0.0:
                    bc = const.tile([P, 1], f32, tag="bc_" + bn)
                    nc.vector.memset(bc[:], c)
                    bconst[bn] = bc
            xT = const.tile([P, XK * NTOK], f16)
            hist = const.tile([P, T * P], f16)
            zeros16 = const.tile([P, P], f16)
            nc.vector.memset(zeros16[:], 0.0)
            zeros32 = const.tile([P, P], f32)
            nc.vector.memset(zeros32[:], 0.0)

            # ---------------- head: gather + transpose ----------------
            with tc.tile_pool(name="gather", bufs=4) as gp, \
                 tc.tile_pool(name="tpsum", bufs=4, space="PSUM") as tp:
                for g in range(NG):
                    xg = gp.tile([P, NU], f16, tag="xg")
                    nc.gpsimd.indirect_dma_start(
                        out=xg[:], out_offset=None, in_=emb[:],
                        in_offset=bass.IndirectOffsetOnAxis(
                            ap=idx_sb[:, g:g + 1], axis=0),
                    )
                    for ki in range(XK):
                        pt = tp.tile([P, P], f16, tag="pt")
                        nc.tensor.transpose(pt[:], xg[:, ki * P:(ki + 1) * P],
                                            ident[:])
                        nc.vector.tensor_copy(
                            xT[:, ki * NTOK + g * P: ki * NTOK + (g + 1) * P],
                            pt[:])

            # ---------------- recurrence ----------------
            def act_call(dst, src, func, bname):
                c = bias_consts[bname]
                if c is not None:
                    b = 0.0 if c == 0.0 else bconst[bname][:, 0:1]
                    nc.scalar.activation(dst[:], src[:], func, bias=b)
                else:
                    bt = bsb[bname]
                    for mt in range(8):
                        nc.scalar.activation(
                            dst[:, mt * BC:(mt + 1) * BC],
                            src[:, mt * BC:(mt + 1) * BC],
                            func, bias=bt[:, mt:mt + 1])

            layer_w = [("w0h", "w0t", "b0h", "b0t"),
                       ("w1h", "w1t", "b1h", "b1t"),
                       ("w2h", "w2t", "b2h", "b2t")]

            with tc.tile_pool(name="ps", bufs=2, space="PSUM") as psp, \
                 tc.tile_pool(name="work", bufs=3) as wkp, \
                 tc.tile_pool(name="state", bufs=3) as stp:
                s16 = zeros16
                s32 = zeros32
                for t in range(T):
                    for l in range(3):
                        wh, wt_, bh, bt_ = layer_w[l]
                        nkt = (XK + KT) if l == 0 else KT
                        xoff = XK if l == 0 else 0
                        ps_h = psp.tile([P, P], f32, tag="ph")
                        ps_t = psp.tile([P, P], f32, tag="pt")
                        for ps, wn in ((ps_h, wh), (ps_t, wt_)):
                            wt = wsb[wn]
                            # accumulation must be contiguous per PSUM
                            # region (kt-inner): interleaved-region groups
                            # produce wrong results on this hw/compiler
                            for mt in range(KT):
                                for kt in range(nkt):
                                    if l == 0 and kt < XK:
                                        rhs = xT[:, kt * NTOK + t * BC:
                                                 kt * NTOK + (t + 1) * BC]
                                    else:
                                        k = kt - xoff
                                        rhs = s16[:, k * BC:(k + 1) * BC]
                                    nc.tensor.matmul(
                                        ps[:, mt * BC:(mt + 1) * BC],
                                        lhsT=wt[:, (kt * KT + mt) * P:
                                                (kt * KT + mt + 1) * P],
                                        rhs=rhs,
                                        start=(kt == 0), stop=(kt == nkt - 1))
                        h32 = wkp.tile([P, P], f32, tag="h32")
                        act_call(h32, ps_h, Act.Tanh, bh)
                        g32 = wkp.tile([P, P], f32, tag="g32")
                        act_call(g32, ps_t, Act.Sigmoid, bt_)
                        d = wkp.tile([P, P], f32, tag="d")
                        nc.vector.tensor_sub(d[:], h32[:], s32[:])
                        ns32 = stp.tile([P, P], f32, tag="s32")
                        nc.vector.tensor_mul(d[:], d[:], g32[:])
                        nc.vector.tensor_add(ns32[:], d[:], s32[:])
                        s32 = ns32
                        if l < 2:
                            ns16 = stp.tile([P, P], f16, tag="s16")
                            nc.vector.tensor_copy(ns16[:], ns32[:])
                            s16 = ns16
                        else:
                            nc.vector.tensor_copy(
                                hist[:, t * P:(t + 1) * P], ns32[:])
                            s16 = hist[:, t * P:(t + 1) * P]

            if DEBUG:
                nc.sync.dma_start(dbg["hist"][:], hist[:])
                nc.sync.dma_start(dbg["xT"][:], xT[:])

            # ---------------- projection ----------------
            histr = hist[:].rearrange("p (t k b) -> p t k b", t=T, k=KT)
            with tc.tile_pool(name="pp", bufs=2, space="PSUM") as pp, \
                 tc.tile_pool(name="ob", bufs=3) as ob:
                CH = 8            # 8 chunks of 32 steps = 512 cols
                TCH = T // CH
                for mt in range(NU // P):
                    for ch in range(CH):
                        po = pp.tile([P, TCH * BC], f32, tag="po")
                        for kt in range(KT):
                            rhs = histr[:, ch * TCH:(ch + 1) * TCH, kt, :]
                            nc.tensor.matmul(
                                po[:],
                                lhsT=wsb["wp"][:, (kt * (NU // P) + mt) * P:
                                               (kt * (NU // P) + mt + 1) * P],
                                rhs=rhs, start=(kt == 0), stop=(kt == KT - 1))
                        o = ob.tile([P, TCH * BC], f32, tag="o")
                        c = bias_consts["bp"]
                        if c is not None:
                            if c == 0.0:
                                nc.scalar.copy(o[:], po[:])
                            else:
                                nc.scalar.activation(o[:], po[:], Act.Identity,
                                                     bias=bconst["bp"][:, 0:1])
                        else:
                            nc.scalar.activation(o[:], po[:], Act.Identity,
                                                 bias=bsb["bp"][:, mt:mt + 1])
                        nc.sync.dma_start(
                            out[mt * P:(mt + 1) * P,
                                ch * TCH * BC:(ch + 1) * TCH * BC], o[:])
    nc.compile()
    return nc


def kernel(input_data, embedding, Wh0x, Wh0s, bh0, Wt0x, Wt0s, bt0,
           Whh, bhh, Wth, bth, Wp, bp):
    from concourse.bass_utils import run_bass_kernel_spmd

    input_data = np.asarray(input_data)
    embedding = np.asarray(embedding, np.float32)

    bias_vals = {
        "b0h": np.asarray(bh0, np.float32), "b0t": np.asarray(bt0, np.float32),
        "b1h": np.asarray(bhh[0], np.float32), "b1t": np.asarray(bth[0], np.float32),
        "b2h": np.asarray(bhh[1], np.float32), "b2t": np.asarray(bth[1], np.float32),
        "bp": np.asarray(bp, np.float32),
    }
    bias_consts = {k: _uniform(v) for k, v in bias_vals.items()}

    key = tuple(sorted((k, v is None) for k, v in bias_consts.items()))
    if key not in _CACHED:
        _CACHED.clear()
        _CACHED[key] = _build_program(bias_consts)
    nc = _CACHED[key]

    emb16 = np.asarray(embedding, np.float16)
    w_common = {
        "emb": emb16,
        "w0h": _tile_layout(np.concatenate([np.asarray(Wh0x, np.float32),
                                            np.asarray(Wh0s, np.float32)], 0)),
        "w0t": _tile_layout(np.concatenate([np.asarray(Wt0x, np.float32),
                                            np.asarray(Wt0s, np.float32)], 0)),
        "w1h": _tile_layout(np.asarray(Whh[0], np.float32)),
        "w1t": _tile_layout(np.asarray(Wth[0], np.float32)),
        "w2h": _tile_layout(np.asarray(Whh[1], np.float32)),
        "w2t": _tile_layout(np.asarray(Wth[1], np.float32)),
        "wp": _tile_layout(np.asarray(Wp, np.float32)),
    }
    for bn, bv in bias_vals.items():
        if bias_consts[bn] is None:
            nt = bv.size // P
            w_common[bn] = np.ascontiguousarray(
                bv.reshape(nt, P).T.astype(np.float32))

    in_maps = []
    for c in range(NCORES):
        ids_c = input_data[c * BC:(c + 1) * BC, :]          # [16, 256]
        ids_tm = ids_c.T.reshape(-1)                        # token j = t*16+b
        idx_c = np.ascontiguousarray(
            ids_tm.reshape(NG, P).T.astype(np.int32))       # [128, 32]
        m = dict(w_common)
        m["idx"] = idx_c
        in_maps.append(m)

    res = run_bass_kernel_spmd(nc, in_maps, list(range(NCORES)))

    out_full = np.empty((B * T, NU), np.float32)
    for c in range(NCORES):
        oc = res.results[c]["out"]                          # [256, 4096]
        oc = oc.reshape(NU, T, BC).transpose(2, 1, 0)       # [b, t, nu]
        out_full[c * BC * T:(c + 1) * BC * T] = oc.reshape(BC * T, NU)
    return out_full
